# revision 12
# baseline (speedup 1.0000x reference)
"""Trainium2 Bass kernel for nn_SSMLayer_17514876633683.

Math: the reference SSM state update broadcasts the input over H and starts
from zero state, so state[b,:,h] is identical for every h.  The whole layer
collapses to:
    z_t[b]    = A @ z_{t-1}[b] + B @ x[b,t]          (z in R^S, S=128)
    c[b,t]    = Cbar . z_t[b]                         (Cbar = C.mean(0))
    y_pre     = c[b,t] + (x @ D.T)[b,t,:]
    y         = LN(gelu(y_pre) + x) * gamma + beta

Sharding: 8 cores = 4 batches x 2 time-halves.  SPMD: every core gets a
(possibly front-zero-padded) sequence and computes output rows 256..511 of
the padded timeline.

Dataflow (v6):
  * x arrives TRANSPOSED (xbt = x^T window) for the matmul path and as
    plain rows (xrows) for the residual; no on-device transposes.
  * c[t] is injected via the per-partition gelu BIAS on ScalarE.
  * Q=4 scan chunks; truncated chunk-lag convolution for the boundary term.
  * Inputs packed into 3 need-ordered DMAs per fast queue (scalar+gpsimd;
    the sync HW queue is ~5x slower).  xD accumulation follows pdt-half
    arrival order.
  * HAM: the PE cold-starts ~4x throttled and un-throttles only after
    ~3.3us of dense matmul activity, so a warmup accumulation group plus
    pinned fillers keep the duty cycle high until the xD burst.
  * ScalarE holds one ACT table set at a time: gelu preload early, the
    sqrt set load pinned AFTER the last gelu (a wrong order costs 1.3us).

Scan mapping (window = last 256+Q*LZ steps of the padded timeline):
  U = B @ x^T                      (S x W)            - 4 PE matmuls
  R_w = sum_r A^(Q-1-r) U[:,wQ+r]  (chunk summaries)  - Q matmuls
  c^T[jj,i] = sum_L g_i . (A^Q)^L R_{63+jj-L}         - LZ matmuls
            + sum_{k<i} g_{i-1-k} . U[..]             - Q matmuls
  c_col[p]  = masked scatter + ones-matmul -> [128,1] gelu bias
All A-power / g weight matrices are precomputed host-side.  Matmul operands
are bf16 (fp32 PSUM accumulation); LN statistics are computed in fp32 by
the DVE from the bf16 sum.
"""

import sys
from contextlib import ExitStack

sys.path.insert(0, "/opt/trn_rl_repo")

import ml_dtypes
import numpy as np

import concourse.bass as bass  # noqa: F401
import concourse.mybir as mybir
import concourse.tile as tile
from concourse import bacc, bass_utils
from concourse.tile_rust import add_dep_helper

# Problem shapes (hardcoded per the harness contract).
BSZ, T, H, S = 4, 512, 512, 128
Q = 4            # scan chunk length
NCH = T // Q     # 128 chunks
TOUT = 256       # output rows per core
LN_EPS = 1e-5
NCORES = 8
NWARM = 9
TRUNC_TOL = 2e-3   # lag truncation: c is ~6% of signal, budget is 2e-2

F32 = mybir.dt.float32
F16 = mybir.dt.float16
BF16 = mybir.dt.bfloat16
BF16_NP = ml_dtypes.bfloat16
AF = mybir.ActivationFunctionType
ALU = mybir.AluOpType


def _host_weights(A, Bm, Cm):
    """Precompute scan weights; returns (APOW, APQL, GW, LZ) float64."""
    A64 = A.astype(np.float64)
    Cbar = Cm.astype(np.float64).mean(axis=0)          # (S,)

    pows = [np.eye(S)]
    for _ in range(Q):
        pows.append(pows[-1] @ A64)                    # pows[k] = A^k
    AQm = pows[Q]

    # lhsT blocks for R: block r holds (A^(Q-1-r))^T
    APOW = np.concatenate([pows[Q - 1 - r].T for r in range(Q)], axis=1)

    # boundary-lag powers, truncated once ||(A^Q)^L|| is negligible
    qp = [np.eye(S)]
    while len(qp) < NCH // 2:
        nxt = qp[-1] @ AQm
        if np.linalg.norm(nxt, 2) < TRUNC_TOL:
            break
        qp.append(nxt)
    LZ = len(qp)

    g = [pows[k].T @ Cbar for k in range(Q)]           # g_k = (A^T)^k Cbar
    GQ = np.stack(g, axis=1)                           # (S, Q)
    APQL = np.concatenate([m.T @ GQ for m in qp], axis=1)  # (S, LZ*Q)
    WTRI = np.zeros((S, Q * Q))
    for k in range(Q):
        for i in range(Q):
            if i > k:
                WTRI[:, k * Q + i] = g[i - 1 - k]
    GW = np.concatenate([GQ, WTRI], axis=1)            # (S, Q + Q*Q)

    return APOW, APQL, GW, LZ


def _emit(tc, aps, apply_gamma_beta, LZ):
    nc = tc.nc
    pka, pkb, pkc, pkd, pke, pkf = (aps["pka"], aps["pkb"], aps["pkc"],
                                    aps["pkd"], aps["pke"], aps["pkf"])
    yout = aps["yout"]
    p32 = aps.get("p32")
    W = TOUT + Q * LZ              # live window columns
    woff = Q * LZ                  # xbt col of first output row
    nchr = TOUT // Q + LZ          # live R chunks
    NJH = NCH // 2                 # output-half chunks (64)
    NB = LZ * Q + Q + Q * Q + 256  # pbtb columns

    ctx = ExitStack()
    cpool = ctx.enter_context(tc.tile_pool(name="const", bufs=1))
    wpool = ctx.enter_context(tc.tile_pool(name="work", bufs=2))
    spp = ctx.enter_context(tc.tile_pool(name="spp", bufs=1, space="PSUM"))
    ypp = ctx.enter_context(tc.tile_pool(name="ypp", bufs=2, space="PSUM"))
    wpp = ctx.enter_context(tc.tile_pool(name="wpp", bufs=1, space="PSUM"))

    # ---- input loads first: 3 need-ordered packs per fast queue -----------
    PKA = cpool.tile([128, 4 * S + 2 * W], BF16, tag="PKA")
    PKB = cpool.tile([128, Q * S + H], BF16, tag="PKB")
    PKC = cpool.tile([128, H], BF16, tag="PKC")
    PKD = cpool.tile([128, 2 * W], BF16, tag="PKD")
    PKE = cpool.tile([128, NB + H], BF16, tag="PKE")
    PKF = cpool.tile([128, 3 * H], BF16, tag="PKF")
    nc.scalar.dma_start(PKA[:], pka)
    nc.scalar.dma_start(PKB[:], pkb)
    nc.scalar.dma_start(PKC[:], pkc)
    nc.gpsimd.dma_start(PKD[:], pkd)
    nc.gpsimd.dma_start(PKE[:], pke)
    nc.gpsimd.dma_start(PKF[:], pkf)
    if apply_gamma_beta:
        P32 = cpool.tile([128, p32.shape[1]], F32, tag="P32")
        nc.sync.dma_start(P32[:], p32)
        gb_sb = P32[:, 0:2 * H].rearrange("p (g h) -> p g h", g=2)

    # views into the packs
    Bt_sb = PKA[:, 0:4 * S].rearrange("p (hh s) -> p hh s", hh=4)
    xbt = [PKA[:, 4 * S:4 * S + W],            # h0
           PKA[:, 4 * S + W:4 * S + 2 * W],    # h1
           PKD[:, 0:W],                        # h2
           PKD[:, W:2 * W]]                    # h3
    APOW_sb = PKB[:, 0:Q * S]
    pdt = [PKB[:, Q * S:Q * S + H],            # h0
           PKC[:, 0:H],                        # h1
           PKE[:, NB:NB + H],                  # h2
           PKF[:, 0:H]]                        # h3
    APQL_sb = PKE[:, 0:LZ * Q]
    GW_sb = PKE[:, LZ * Q:LZ * Q + Q + Q * Q]
    o_ms = LZ * Q + Q + Q * Q
    xrows = [PKF[:, H:2 * H], PKF[:, 2 * H:3 * H]]

    # ---- small consts + act-table preload (gelu) --------------------------
    # memsets on the DVE: the gpsimd sequencer is busy pushing DMAs and a
    # late warm_sb memset delays the whole HAM warmup chain.
    warm_sb = cpool.tile([128, 256], BF16, tag="warm_sb")
    nc.vector.memset(warm_sb[:], 0.0)
    eps_sb = cpool.tile([128, 1], F32, tag="eps_sb")
    nc.vector.memset(eps_sb[:], LN_EPS)
    ones4 = cpool.tile([NJH, 1], BF16, tag="ones4")
    nc.vector.memset(ones4[:], 1.0)
    gsc = cpool.tile([128, 1], F32, tag="gsc")
    nc.vector.memset(gsc[:], 0.0)
    nc.scalar.activation(gsc[:], gsc[:], AF.Gelu)

    # ---- PE warmup: one accumulation group + pinned fillers ---------------
    # Dense matmul activity trips the HAM un-throttle (cold PE runs ~4x
    # slow); keep the duty cycle high until the xD burst is done.
    wp = wpp.tile([128, 256], F32, tag="warm_ps", name="warm_ps")
    for i in range(NWARM):
        nc.tensor.matmul(wp[:], lhsT=warm_sb[:, :128], rhs=warm_sb[:],
                         start=(i == 0), stop=(i == NWARM - 1))

    def fillers(n, before=None):
        # pinned fillers run BEFORE `before` retires: they bridge PE idle
        # gaps (DMA waits, PSUM->SBUF copies) to keep the HAM credit alive
        for _ in range(n):
            mi = nc.tensor.matmul(wp[:], lhsT=warm_sb[:, :128], rhs=warm_sb[:],
                                  start=True, stop=True)
            if before is not None:
                add_dep_helper(before, mi.ins, False, "pin filler")

    # ---- U = B @ x^T over the live window (S x W) -------------------------
    # All small scan-stage PSUM tiles share one 2KB bank.
    scan_ps = spp.tile([128, 512], F32, tag="scan")
    U_ps = scan_ps[:, 0:W]
    u_mms = []
    for hh in range(4):
        u_mms.append(nc.tensor.matmul(U_ps[:], lhsT=Bt_sb[:, hh, :],
                                      rhs=xbt[hh], start=(hh == 0),
                                      stop=(hh == 3)))
    fillers(2, before=u_mms[0].ins)
    U_sb = cpool.tile([128, W], BF16, tag="U_sb")
    U_sb3 = U_sb.rearrange("s (r w) -> s r w", r=Q)    # r-major store
    U_ps3 = U_ps.rearrange("s (w r) -> s r w", r=Q)
    nc.vector.tensor_copy(U_sb3[:], U_ps3[:])
    U_r = U_sb3                                        # [128, Q, nchr]

    # ---- chunk summaries R ------------------------------------------------
    R_ps = scan_ps[:, W:W + nchr]
    r_mms = []
    for r in range(Q):
        r_mms.append(nc.tensor.matmul(R_ps[:],
                                      lhsT=APOW_sb[:, r * S:(r + 1) * S],
                                      rhs=U_r[:, r, :], start=(r == 0),
                                      stop=(r == Q - 1)))
    fillers(2, before=r_mms[0].ins)
    R_sb = cpool.tile([128, nchr], BF16, tag="R_sb")
    nc.vector.tensor_copy(R_sb[:], R_ps[:])

    # ---- c^T for the output half (jj in [0,64), i in [0,4)) ---------------
    c_psT = scan_ps[0:NJH, W + nchr:W + nchr + Q]
    c_first = None
    for L in range(LZ):
        mi = nc.tensor.matmul(c_psT[:],
                              lhsT=R_sb[:, LZ - 1 - L:LZ - 1 - L + NJH],
                              rhs=APQL_sb[:, L * Q:(L + 1) * Q],
                              start=(L == 0), stop=False)
        if L == 0:
            c_first = mi
    for k in range(Q):
        nc.tensor.matmul(
            c_psT[:], lhsT=U_r[:, k, LZ:LZ + NJH],
            rhs=GW_sb[:, Q + k * Q:Q + (k + 1) * Q],
            start=False, stop=(k == Q - 1))
    fillers(1, before=c_first.ins)

    # ---- scatter c into per-row lhsT columns, then ones-matmul ------------
    # lhsTc_n[j, p] = c^T[j, p%Q] * [j == 32n + p//Q]; c_col_n[p] = c[128n+p]
    c_bc = c_psT[:, None, :].to_broadcast((NJH, 128 // Q, Q))
    c_col = scan_ps[:, W + nchr + Q:W + nchr + Q + 2]
    lhsTcs = []
    for n in range(2):
        msk = PKE[0:NJH, o_ms + n * 128:o_ms + (n + 1) * 128]
        lhsTc = cpool.tile([NJH, 128], BF16, tag=f"lhsTc{n}",
                           name=f"lhsTc{n}")
        nc.vector.tensor_tensor(
            lhsTc.rearrange("j (jm i) -> j jm i", jm=128 // Q), c_bc,
            msk.rearrange("j (jm i) -> j jm i", jm=128 // Q), ALU.mult)
        lhsTcs.append(lhsTc)
    for n in range(2):
        nc.tensor.matmul(c_col[:, n:n + 1], lhsT=lhsTcs[n][:], rhs=ones4[:],
                         start=True, stop=True)
    c_sb = cpool.tile([128, 2], F32, tag="c_sb")
    nc.vector.tensor_copy(c_sb[:], c_col[:])

    # ---- xD into two PSUM tiles, hh in DMA-arrival order ------------------
    y_pss = []
    for tt2 in range(2):
        y_pss.append(ypp.tile([128, H], F32, tag="y_ps", name=f"y_ps{tt2}"))
    harr = [0, 2, 1, 3]            # pack order: B, E, C, F
    for i, hh in enumerate(harr):
        for tt2 in range(2):
            nc.tensor.matmul(
                y_pss[tt2][:],
                lhsT=xbt[hh][:, woff + tt2 * 128:woff + (tt2 + 1) * 128],
                rhs=pdt[hh], start=(i == 0), stop=(i == 3))

    # ---- gelu(y + c) + residual + stats (bf16 tail on the DVE) ------------
    y_sbs, mvs, gelus = [], [], []
    for tt2 in range(2):
        g_sb = wpool.tile([128, H], BF16, tag="g_sb", name=f"g_sb{tt2}")
        gi = nc.scalar.activation(g_sb[:], y_pss[tt2][:], AF.Gelu,
                                  bias=c_sb[:, tt2:tt2 + 1], scale=1.0)
        gelus.append(gi)
        y_sb = wpool.tile([128, H], BF16, tag=f"y_sb{tt2}", name=f"y_sb{tt2}")
        nc.vector.tensor_add(y_sb[:], g_sb[:], xrows[tt2])
        st6 = wpool.tile([128, 6], F32, tag="st6", name=f"st6_{tt2}")
        nc.vector.bn_stats(st6[:], y_sb[:])
        mv = wpool.tile([128, 2], F32, tag=f"mv{tt2}", name=f"mv{tt2}")
        nc.vector.bn_aggr(mv[:], st6[:])
        y_sbs.append(y_sb)
        mvs.append(mv)

    # Sqrt ACT-table load pinned AFTER the last gelu (the engine holds one
    # table set at a time; loading earlier would evict the gelu table).
    rsc = wpool.tile([128, 1], F32, tag="rsc")
    ri = nc.scalar.activation(rsc[:], eps_sb[:], AF.Sqrt, bias=eps_sb[:],
                              scale=1.0)
    add_dep_helper(ri.ins, gelus[1].ins, False, "sqrt table after gelus")

    # ---- normalize and write out ------------------------------------------
    for tt2 in range(2):
        y_sb, mv = y_sbs[tt2], mvs[tt2]
        sd = wpool.tile([128, 1], F32, tag=f"sd{tt2}", name=f"sd{tt2}")
        nc.scalar.activation(sd[:], mv[:, 1:2], AF.Sqrt, bias=eps_sb[:],
                             scale=1.0)
        iv = wpool.tile([128, 1], F32, tag=f"iv{tt2}", name=f"iv{tt2}")
        nc.vector.reciprocal(iv[:], sd[:])
        o_sb = wpool.tile([128, H], F16, tag="o_sb", name=f"o_sb{tt2}")
        nc.vector.tensor_scalar(o_sb[:], y_sb[:], mv[:, 0:1], iv[:],
                                op0=ALU.subtract, op1=ALU.mult)
        if apply_gamma_beta:
            nc.vector.tensor_tensor(o_sb[:], o_sb[:], gb_sb[:, 0, :], ALU.mult)
            nc.vector.tensor_tensor(o_sb[:], o_sb[:], gb_sb[:, 1, :], ALU.add)
        out_eng = nc.scalar if tt2 == 0 else nc.gpsimd
        out_eng.dma_start(yout[tt2 * 128:(tt2 + 1) * 128, :], o_sb[:])

    ctx.close()


def _build_program(apply_gamma_beta, LZ):
    nc = bacc.Bacc("TRN2", target_bir_lowering=False, debug=False,
                   enable_asserts=False, num_devices=NCORES)
    W = TOUT + Q * LZ
    NB = LZ * Q + Q + Q * Q + 256
    shapes = {
        "pka": 4 * S + 2 * W,
        "pkb": Q * S + H,
        "pkc": H,
        "pkd": 2 * W,
        "pke": NB + H,
        "pkf": 3 * H,
    }
    aps = {k: nc.dram_tensor(k, (128, n), BF16, kind="ExternalInput").ap()
           for k, n in shapes.items()}
    aps["yout"] = nc.dram_tensor("yout", (TOUT, H), F16,
                                 kind="ExternalOutput").ap()
    if apply_gamma_beta:
        aps["p32"] = nc.dram_tensor("p32", (128, 2 * H), F32,
                                    kind="ExternalInput").ap()
    with tile.TileContext(nc) as tc:
        _emit(tc, aps, apply_gamma_beta, LZ)
    nc.compile()
    return nc


def _prepare_in_maps(x, A, Bm, Cm, D, gamma, beta, apply_gamma_beta):
    APOW, APQL, GW, LZ = _host_weights(A, Bm, Cm)
    W = TOUT + Q * LZ
    t0 = T - W                      # window start in padded timeline

    def part_major(m, inner):
        # (4*128, inner) -> (128, 4*inner):  row (hh*128+p) -> [p, hh*inner:]
        return np.ascontiguousarray(
            m.reshape(4, 128, inner).transpose(1, 0, 2).reshape(128, 4 * inner))

    msk = np.zeros((128, 256))
    for n in range(2):
        for p in range(128):
            msk[32 * n + p // Q, n * 128 + p] = 1.0
    Bt = part_major(Bm.T, S)                           # (128, 4*S)
    Dt = part_major(D.T, H)                            # (128, 4*H)
    pbtb = np.concatenate([APQL, GW, msk], axis=1)     # (128, NB)

    in_maps = []
    for core in range(NCORES):
        b, half = core // 2, core % 2
        if half == 0:
            xp = np.concatenate(
                [np.zeros((TOUT, H), np.float32), x[b, :TOUT]], axis=0)
        else:
            xp = x[b]
        xbt = part_major(np.ascontiguousarray(xp[t0:].T), W)  # (128, 4*W)
        xh = [xbt[:, i * W:(i + 1) * W] for i in range(4)]
        dh = [Dt[:, i * H:(i + 1) * H] for i in range(4)]
        xr = xp[TOUT:].reshape(2, 128, H).transpose(1, 0, 2)  # (128, 2, H)
        m = {
            "pka": np.concatenate([Bt, xh[0], xh[1]], axis=1),
            "pkb": np.concatenate([APOW, dh[0]], axis=1),
            "pkc": dh[1],
            "pkd": np.concatenate([xh[2], xh[3]], axis=1),
            "pke": np.concatenate([pbtb, dh[2]], axis=1),
            "pkf": np.concatenate([dh[3], xr[:, 0], xr[:, 1]], axis=1),
        }
        m = {k: np.ascontiguousarray(v).astype(BF16_NP) for k, v in m.items()}
        if apply_gamma_beta:
            p32 = [np.broadcast_to(gamma, (128, H)),
                   np.broadcast_to(beta, (128, H))]
            m["p32"] = np.ascontiguousarray(
                np.concatenate(p32, axis=1).astype(np.float32))
        in_maps.append(m)
    return in_maps, LZ


def _run(inputs, trace=False):
    x = np.asarray(inputs["x"], np.float32)
    A = np.asarray(inputs["A"], np.float32)
    Bm = np.asarray(inputs["B"], np.float32)
    Cm = np.asarray(inputs["C"], np.float32)
    D = np.asarray(inputs["D"], np.float32)
    gamma = np.asarray(inputs["gamma"], np.float32)
    beta = np.asarray(inputs["beta"], np.float32)

    apply_gamma_beta = not (np.all(gamma == 1.0) and np.all(beta == 0.0))
    in_maps, LZ = _prepare_in_maps(x, A, Bm, Cm, D, gamma, beta,
                                   apply_gamma_beta)
    nc = _build_program(apply_gamma_beta, LZ)
    res = bass_utils.run_bass_kernel_spmd(
        nc, in_maps, core_ids=list(range(NCORES)), trace=trace)
    y = np.empty((BSZ, T, H), np.float32)
    for core in range(NCORES):
        b, half = core // 2, core % 2
        y[b, half * TOUT:(half + 1) * TOUT, :] = (
            res.results[core]["yout"].astype(np.float32))
    return y, res


def kernel(**inputs):
    y, _ = _run(inputs, trace=False)
    return y


def kernel_traced(**inputs):
    return _run(inputs, trace=True)


# revision 13
# speedup vs baseline: 1.1186x; 1.1186x over previous
"""Trainium2 Bass kernel for nn_SSMLayer_17514876633683.

Math: the reference SSM state update broadcasts the input over H and starts
from zero state, so state[b,:,h] is identical for every h.  The whole layer
collapses to:
    z_t[b]    = A @ z_{t-1}[b] + B @ x[b,t]          (z in R^S, S=128)
    c[b,t]    = Cbar . z_t[b]                         (Cbar = C.mean(0))
    y_pre     = c[b,t] + (x @ D.T)[b,t,:]
    y         = LN(gelu(y_pre) + x) * gamma + beta

Sharding: 8 cores = 4 batches x 2 time-halves.  SPMD: every core gets a
(possibly front-zero-padded) sequence and computes output rows 256..511 of
the padded timeline.

Dataflow (v6):
  * x arrives TRANSPOSED (xbt = x^T window) for the matmul path and as
    plain rows (xrows) for the residual; no on-device transposes.
  * c[t] is injected via the per-partition gelu BIAS on ScalarE.
  * Q=4 scan chunks; truncated chunk-lag convolution for the boundary term.
  * Inputs packed into 3 need-ordered DMAs per fast queue (scalar+gpsimd;
    the sync HW queue is ~5x slower).  xD accumulation follows pdt-half
    arrival order.
  * HAM: the PE cold-starts ~4x throttled and un-throttles only after
    ~3.3us of dense matmul activity, so a warmup accumulation group plus
    pinned fillers keep the duty cycle high until the xD burst.
  * ScalarE holds one ACT table set at a time: gelu preload early, the
    sqrt set load pinned AFTER the last gelu (a wrong order costs 1.3us).

Scan mapping (window = last 256+Q*LZ steps of the padded timeline):
  U = B @ x^T                      (S x W)            - 4 PE matmuls
  R_w = sum_r A^(Q-1-r) U[:,wQ+r]  (chunk summaries)  - Q matmuls
  c^T[jj,i] = sum_L g_i . (A^Q)^L R_{63+jj-L}         - LZ matmuls
            + sum_{k<i} g_{i-1-k} . U[..]             - Q matmuls
  c_col[p]  = masked scatter + ones-matmul -> [128,1] gelu bias
All A-power / g weight matrices are precomputed host-side.  Matmul operands
are bf16 (fp32 PSUM accumulation); LN statistics are computed in fp32 by
the DVE from the bf16 sum.
"""

import sys
from contextlib import ExitStack

sys.path.insert(0, "/opt/trn_rl_repo")

import ml_dtypes
import numpy as np

import concourse.bass as bass  # noqa: F401
import concourse.mybir as mybir
import concourse.tile as tile
from concourse import bacc, bass_utils
from concourse.tile_rust import add_dep_helper

# Problem shapes (hardcoded per the harness contract).
BSZ, T, H, S = 4, 512, 512, 128
Q = 4            # scan chunk length
NCH = T // Q     # 128 chunks
TOUT = 256       # output rows per core
LN_EPS = 1e-5
NCORES = 8
NWARM = 9
TRUNC_TOL = 2e-3   # lag truncation: c is ~6% of signal, budget is 2e-2

F32 = mybir.dt.float32
F16 = mybir.dt.float16
BF16 = mybir.dt.bfloat16
BF16_NP = ml_dtypes.bfloat16
AF = mybir.ActivationFunctionType
ALU = mybir.AluOpType


def _host_weights(A, Bm, Cm):
    """Precompute scan weights; returns (APOW, APQL, GW, LZ) float64."""
    A64 = A.astype(np.float64)
    Cbar = Cm.astype(np.float64).mean(axis=0)          # (S,)

    pows = [np.eye(S)]
    for _ in range(Q):
        pows.append(pows[-1] @ A64)                    # pows[k] = A^k
    AQm = pows[Q]

    # lhsT blocks for R: block r holds (A^(Q-1-r))^T
    APOW = np.concatenate([pows[Q - 1 - r].T for r in range(Q)], axis=1)

    # boundary-lag powers, truncated once ||(A^Q)^L|| is negligible
    qp = [np.eye(S)]
    while len(qp) < NCH // 2:
        nxt = qp[-1] @ AQm
        if np.linalg.norm(nxt, 2) < TRUNC_TOL:
            break
        qp.append(nxt)
    LZ = len(qp)

    g = [pows[k].T @ Cbar for k in range(Q)]           # g_k = (A^T)^k Cbar
    GQ = np.stack(g, axis=1)                           # (S, Q)
    APQL = np.concatenate([m.T @ GQ for m in qp], axis=1)  # (S, LZ*Q)
    WTRI = np.zeros((S, Q * Q))
    for k in range(Q):
        for i in range(Q):
            if i > k:
                WTRI[:, k * Q + i] = g[i - 1 - k]
    GW = np.concatenate([GQ, WTRI], axis=1)            # (S, Q + Q*Q)

    return APOW, APQL, GW, LZ


def _emit(tc, aps, apply_gamma_beta, LZ):
    nc = tc.nc
    pka, pkb, pkc, pkd, pke, pkf = (aps["pka"], aps["pkb"], aps["pkc"],
                                    aps["pkd"], aps["pke"], aps["pkf"])
    yout = aps["yout"]
    p32 = aps.get("p32")
    W = TOUT + Q * LZ              # live window columns
    woff = Q * LZ                  # xbt col of first output row
    nchr = TOUT // Q + LZ          # live R chunks
    NJH = NCH // 2                 # output-half chunks (64)
    NB = LZ * Q + Q + Q * Q + 256  # pbtb columns

    ctx = ExitStack()
    cpool = ctx.enter_context(tc.tile_pool(name="const", bufs=1))
    wpool = ctx.enter_context(tc.tile_pool(name="work", bufs=2))
    spp = ctx.enter_context(tc.tile_pool(name="spp", bufs=1, space="PSUM"))
    ypp = ctx.enter_context(tc.tile_pool(name="ypp", bufs=2, space="PSUM"))
    wpp = ctx.enter_context(tc.tile_pool(name="wpp", bufs=1, space="PSUM"))

    # ---- input loads first: 3 need-ordered packs per fast queue -----------
    NS = LZ * Q + Q + Q * Q        # APQL+GW cols ("smalls")
    PKA = cpool.tile([128, 4 * S + 2 * W], BF16, tag="PKA")
    PKB = cpool.tile([128, Q * S + H], BF16, tag="PKB")
    PKC = cpool.tile([128, 2 * H], BF16, tag="PKC")
    PKD = cpool.tile([128, 2 * W + NS], BF16, tag="PKD")
    PKE = cpool.tile([128, 2 * H], BF16, tag="PKE")
    PKF = cpool.tile([128, H], BF16, tag="PKF")
    nc.gpsimd.dma_start(PKA[:], pka)
    nc.gpsimd.dma_start(PKB[:], pkb)
    nc.gpsimd.dma_start(PKC[:], pkc)
    nc.scalar.dma_start(PKD[:], pkd)
    nc.scalar.dma_start(PKE[:], pke)
    nc.scalar.dma_start(PKF[:], pkf)
    if apply_gamma_beta:
        P32 = cpool.tile([128, p32.shape[1]], F32, tag="P32")
        nc.sync.dma_start(P32[:], p32)
        gb_sb = P32[:, 0:2 * H].rearrange("p (g h) -> p g h", g=2)

    # views into the packs
    Bt_sb = PKA[:, 0:4 * S].rearrange("p (hh s) -> p hh s", hh=4)
    xbt = [PKA[:, 4 * S:4 * S + W],            # h0
           PKA[:, 4 * S + W:4 * S + 2 * W],    # h1
           PKD[:, 0:W],                        # h2
           PKD[:, W:2 * W]]                    # h3
    APOW_sb = PKB[:, 0:Q * S]
    pdt = [PKB[:, Q * S:Q * S + H],            # h0
           PKC[:, 0:H],                        # h1
           PKE[:, 0:H],                        # h2
           PKE[:, H:2 * H]]                    # h3
    APQL_sb = PKD[:, 2 * W:2 * W + LZ * Q]
    GW_sb = PKD[:, 2 * W + LZ * Q:2 * W + NS]
    xrows = [PKC[:, H:2 * H], PKF[:, 0:H]]

    # scatter masks built on device (saves 64KB of DMA):
    # msk_n[j, jm, i] = 1 iff jm == j - 32n   (j<64, jm<32, i<4)
    msks = []
    for n in range(2):
        mt = cpool.tile([NJH, 128], BF16, tag=f"msk{n}", name=f"msk{n}")
        nc.gpsimd.memset(mt[:], 0.0)
        nc.gpsimd.affine_select(
            out=mt.rearrange("j (jm i) -> j jm i", jm=128 // Q),
            in_=mt.rearrange("j (jm i) -> j jm i", jm=128 // Q),
            compare_op=ALU.not_equal, fill=1.0, base=-32 * n,
            pattern=[[-1, 128 // Q], [0, Q]], channel_multiplier=1)
        msks.append(mt)

    # ---- small consts + act-table preload (gelu) --------------------------
    # memsets on the DVE: the gpsimd sequencer is busy pushing DMAs and a
    # late warm_sb memset delays the whole HAM warmup chain.
    warm_sb = cpool.tile([128, 256], BF16, tag="warm_sb")
    nc.vector.memset(warm_sb[:], 0.0)
    eps_sb = cpool.tile([128, 1], F32, tag="eps_sb")
    nc.vector.memset(eps_sb[:], LN_EPS)
    ones4 = cpool.tile([NJH, 1], BF16, tag="ones4")
    nc.vector.memset(ones4[:], 1.0)
    gsc = cpool.tile([128, 1], F32, tag="gsc")
    nc.vector.memset(gsc[:], 0.0)
    nc.scalar.activation(gsc[:], gsc[:], AF.Gelu)

    # ---- PE warmup: one accumulation group + pinned fillers ---------------
    # Dense matmul activity trips the HAM un-throttle (cold PE runs ~4x
    # slow); keep the duty cycle high until the xD burst is done.
    wp = wpp.tile([128, 256], F32, tag="warm_ps", name="warm_ps")
    for i in range(NWARM):
        nc.tensor.matmul(wp[:], lhsT=warm_sb[:, :128], rhs=warm_sb[:],
                         start=(i == 0), stop=(i == NWARM - 1))

    def fillers(n, before=None):
        # pinned fillers run BEFORE `before` retires: they bridge PE idle
        # gaps (DMA waits, PSUM->SBUF copies) to keep the HAM credit alive
        for _ in range(n):
            mi = nc.tensor.matmul(wp[:], lhsT=warm_sb[:, :128], rhs=warm_sb[:],
                                  start=True, stop=True)
            if before is not None:
                add_dep_helper(before, mi.ins, False, "pin filler")

    # ---- U = B @ x^T over the live window (S x W) -------------------------
    # All small scan-stage PSUM tiles share one 2KB bank.
    scan_ps = spp.tile([128, 512], F32, tag="scan")
    U_ps = scan_ps[:, 0:W]
    u_mms = []
    for hh in range(4):
        u_mms.append(nc.tensor.matmul(U_ps[:], lhsT=Bt_sb[:, hh, :],
                                      rhs=xbt[hh], start=(hh == 0),
                                      stop=(hh == 3)))
    fillers(2, before=u_mms[0].ins)
    U_sb = cpool.tile([128, W], BF16, tag="U_sb")
    U_sb3 = U_sb.rearrange("s (r w) -> s r w", r=Q)    # r-major store
    U_ps3 = U_ps.rearrange("s (w r) -> s r w", r=Q)
    nc.vector.tensor_copy(U_sb3[:], U_ps3[:])
    U_r = U_sb3                                        # [128, Q, nchr]

    # ---- chunk summaries R ------------------------------------------------
    R_ps = scan_ps[:, W:W + nchr]
    r_mms = []
    for r in range(Q):
        r_mms.append(nc.tensor.matmul(R_ps[:],
                                      lhsT=APOW_sb[:, r * S:(r + 1) * S],
                                      rhs=U_r[:, r, :], start=(r == 0),
                                      stop=(r == Q - 1)))
    fillers(2, before=r_mms[0].ins)
    R_sb = cpool.tile([128, nchr], BF16, tag="R_sb")
    nc.vector.tensor_copy(R_sb[:], R_ps[:])

    # ---- c^T for the output half (jj in [0,64), i in [0,4)) ---------------
    c_psT = scan_ps[0:NJH, W + nchr:W + nchr + Q]
    c_first = None
    for L in range(LZ):
        mi = nc.tensor.matmul(c_psT[:],
                              lhsT=R_sb[:, LZ - 1 - L:LZ - 1 - L + NJH],
                              rhs=APQL_sb[:, L * Q:(L + 1) * Q],
                              start=(L == 0), stop=False)
        if L == 0:
            c_first = mi
    for k in range(Q):
        nc.tensor.matmul(
            c_psT[:], lhsT=U_r[:, k, LZ:LZ + NJH],
            rhs=GW_sb[:, Q + k * Q:Q + (k + 1) * Q],
            start=False, stop=(k == Q - 1))
    fillers(1, before=c_first.ins)

    # ---- scatter c into per-row lhsT columns, then ones-matmul ------------
    # lhsTc_n[j, p] = c^T[j, p%Q] * [j == 32n + p//Q]; c_col_n[p] = c[128n+p]
    c_bc = c_psT[:, None, :].to_broadcast((NJH, 128 // Q, Q))
    c_col = scan_ps[:, W + nchr + Q:W + nchr + Q + 2]
    lhsTcs = []
    for n in range(2):
        lhsTc = cpool.tile([NJH, 128], BF16, tag=f"lhsTc{n}",
                           name=f"lhsTc{n}")
        nc.vector.tensor_tensor(
            lhsTc.rearrange("j (jm i) -> j jm i", jm=128 // Q), c_bc,
            msks[n].rearrange("j (jm i) -> j jm i", jm=128 // Q), ALU.mult)
        lhsTcs.append(lhsTc)
    for n in range(2):
        nc.tensor.matmul(c_col[:, n:n + 1], lhsT=lhsTcs[n][:], rhs=ones4[:],
                         start=True, stop=True)
    c_sb = cpool.tile([128, 2], F32, tag="c_sb")
    nc.vector.tensor_copy(c_sb[:], c_col[:])

    # ---- xD into two PSUM tiles, hh in DMA-arrival order ------------------
    y_pss = []
    for tt2 in range(2):
        y_pss.append(ypp.tile([128, H], F32, tag="y_ps", name=f"y_ps{tt2}"))
    harr = [0, 2, 3, 1]            # pdt pack arrival: B, E, E, C
    for i, hh in enumerate(harr):
        for tt2 in range(2):
            nc.tensor.matmul(
                y_pss[tt2][:],
                lhsT=xbt[hh][:, woff + tt2 * 128:woff + (tt2 + 1) * 128],
                rhs=pdt[hh], start=(i == 0), stop=(i == 3))

    # ---- gelu(y + c) + residual + stats (bf16 tail on the DVE) ------------
    y_sbs, mvs, gelus = [], [], []
    for tt2 in range(2):
        g_sb = wpool.tile([128, H], BF16, tag="g_sb", name=f"g_sb{tt2}")
        gi = nc.scalar.activation(g_sb[:], y_pss[tt2][:], AF.Gelu,
                                  bias=c_sb[:, tt2:tt2 + 1], scale=1.0)
        gelus.append(gi)
        y_sb = wpool.tile([128, H], BF16, tag=f"y_sb{tt2}", name=f"y_sb{tt2}")
        nc.vector.tensor_add(y_sb[:], g_sb[:], xrows[tt2])
        st6 = wpool.tile([128, 6], F32, tag="st6", name=f"st6_{tt2}")
        nc.vector.bn_stats(st6[:], y_sb[:])
        mv = wpool.tile([128, 2], F32, tag=f"mv{tt2}", name=f"mv{tt2}")
        nc.vector.bn_aggr(mv[:], st6[:])
        y_sbs.append(y_sb)
        mvs.append(mv)

    # Sqrt ACT-table load pinned AFTER the last gelu (the engine holds one
    # table set at a time; loading earlier would evict the gelu table).
    rsc = wpool.tile([128, 1], F32, tag="rsc")
    ri = nc.scalar.activation(rsc[:], eps_sb[:], AF.Sqrt, bias=eps_sb[:],
                              scale=1.0)
    add_dep_helper(ri.ins, gelus[1].ins, False, "sqrt table after gelus")

    # ---- normalize and write out ------------------------------------------
    for tt2 in range(2):
        y_sb, mv = y_sbs[tt2], mvs[tt2]
        sd = wpool.tile([128, 1], F32, tag=f"sd{tt2}", name=f"sd{tt2}")
        nc.scalar.activation(sd[:], mv[:, 1:2], AF.Sqrt, bias=eps_sb[:],
                             scale=1.0)
        iv = wpool.tile([128, 1], F32, tag=f"iv{tt2}", name=f"iv{tt2}")
        nc.vector.reciprocal(iv[:], sd[:])
        o_sb = wpool.tile([128, H], F16, tag="o_sb", name=f"o_sb{tt2}")
        nc.vector.tensor_scalar(o_sb[:], y_sb[:], mv[:, 0:1], iv[:],
                                op0=ALU.subtract, op1=ALU.mult)
        if apply_gamma_beta:
            nc.vector.tensor_tensor(o_sb[:], o_sb[:], gb_sb[:, 0, :], ALU.mult)
            nc.vector.tensor_tensor(o_sb[:], o_sb[:], gb_sb[:, 1, :], ALU.add)
        out_eng = nc.scalar if tt2 == 0 else nc.gpsimd
        out_eng.dma_start(yout[tt2 * 128:(tt2 + 1) * 128, :], o_sb[:])

    ctx.close()


def _build_program(apply_gamma_beta, LZ):
    nc = bacc.Bacc("TRN2", target_bir_lowering=False, debug=False,
                   enable_asserts=False, num_devices=NCORES)
    W = TOUT + Q * LZ
    NS = LZ * Q + Q + Q * Q
    shapes = {
        "pka": 4 * S + 2 * W,
        "pkb": Q * S + H,
        "pkc": 2 * H,
        "pkd": 2 * W + NS,
        "pke": 2 * H,
        "pkf": H,
    }
    aps = {k: nc.dram_tensor(k, (128, n), BF16, kind="ExternalInput").ap()
           for k, n in shapes.items()}
    aps["yout"] = nc.dram_tensor("yout", (TOUT, H), F16,
                                 kind="ExternalOutput").ap()
    if apply_gamma_beta:
        aps["p32"] = nc.dram_tensor("p32", (128, 2 * H), F32,
                                    kind="ExternalInput").ap()
    with tile.TileContext(nc) as tc:
        _emit(tc, aps, apply_gamma_beta, LZ)
    nc.compile()
    return nc


def _prepare_in_maps(x, A, Bm, Cm, D, gamma, beta, apply_gamma_beta):
    APOW, APQL, GW, LZ = _host_weights(A, Bm, Cm)
    W = TOUT + Q * LZ
    t0 = T - W                      # window start in padded timeline

    def part_major(m, inner):
        # (4*128, inner) -> (128, 4*inner):  row (hh*128+p) -> [p, hh*inner:]
        return np.ascontiguousarray(
            m.reshape(4, 128, inner).transpose(1, 0, 2).reshape(128, 4 * inner))

    Bt = part_major(Bm.T, S)                           # (128, 4*S)
    Dt = part_major(D.T, H)                            # (128, 4*H)
    smalls = np.concatenate([APQL, GW], axis=1)        # (128, NS)

    in_maps = []
    for core in range(NCORES):
        b, half = core // 2, core % 2
        if half == 0:
            xp = np.concatenate(
                [np.zeros((TOUT, H), np.float32), x[b, :TOUT]], axis=0)
        else:
            xp = x[b]
        xbt = part_major(np.ascontiguousarray(xp[t0:].T), W)  # (128, 4*W)
        xh = [xbt[:, i * W:(i + 1) * W] for i in range(4)]
        dh = [Dt[:, i * H:(i + 1) * H] for i in range(4)]
        xr = xp[TOUT:].reshape(2, 128, H).transpose(1, 0, 2)  # (128, 2, H)
        m = {
            "pka": np.concatenate([Bt, xh[0], xh[1]], axis=1),
            "pkb": np.concatenate([APOW, dh[0]], axis=1),
            "pkc": np.concatenate([dh[1], xr[:, 0]], axis=1),
            "pkd": np.concatenate([xh[2], xh[3], smalls], axis=1),
            "pke": np.concatenate([dh[2], dh[3]], axis=1),
            "pkf": xr[:, 1],
        }
        m = {k: np.ascontiguousarray(v).astype(BF16_NP) for k, v in m.items()}
        if apply_gamma_beta:
            p32 = [np.broadcast_to(gamma, (128, H)),
                   np.broadcast_to(beta, (128, H))]
            m["p32"] = np.ascontiguousarray(
                np.concatenate(p32, axis=1).astype(np.float32))
        in_maps.append(m)
    return in_maps, LZ


def _run(inputs, trace=False):
    x = np.asarray(inputs["x"], np.float32)
    A = np.asarray(inputs["A"], np.float32)
    Bm = np.asarray(inputs["B"], np.float32)
    Cm = np.asarray(inputs["C"], np.float32)
    D = np.asarray(inputs["D"], np.float32)
    gamma = np.asarray(inputs["gamma"], np.float32)
    beta = np.asarray(inputs["beta"], np.float32)

    apply_gamma_beta = not (np.all(gamma == 1.0) and np.all(beta == 0.0))
    in_maps, LZ = _prepare_in_maps(x, A, Bm, Cm, D, gamma, beta,
                                   apply_gamma_beta)
    nc = _build_program(apply_gamma_beta, LZ)
    res = bass_utils.run_bass_kernel_spmd(
        nc, in_maps, core_ids=list(range(NCORES)), trace=trace)
    y = np.empty((BSZ, T, H), np.float32)
    for core in range(NCORES):
        b, half = core // 2, core % 2
        y[b, half * TOUT:(half + 1) * TOUT, :] = (
            res.results[core]["yout"].astype(np.float32))
    return y, res


def kernel(**inputs):
    y, _ = _run(inputs, trace=False)
    return y


def kernel_traced(**inputs):
    return _run(inputs, trace=True)
